# revision 11
# baseline (speedup 1.0000x reference)
#!/usr/bin/env python3
"""
Trainium2 Bass kernel for nn_AttentionLayer (2-relation GNN edge-softmax
attention message passing), 8 NeuronCores, SPMD.

Sharding: nodes are split into 8 contiguous ranges of 25000 (dst-based edge
parallelism). Core i processes all edges (both relations) whose destination
falls in its range, so per-destination segment softmax/sums complete locally
with no cross-core reduction. Output rows are produced per-core and
concatenated on the host.

Per relation r the math is refactored as
    k = feat @ Ak + ck,  q = feat @ Aq + cq  (Aq, cq pre-scaled by HF^-0.5)
    v = feat @ Av + cv           with A* = W* @ We_r etc. (host-folded)
    logit_e,h = <(k[src]+ck), (q[dst]+cq)>_h
    p = exp(logit)               (no max subtraction: |logit| < ~0.01)
    upd = segsum(p * v[src]) / segsum(p) + cv  (cv folded into bias;
                                               edgeless nodes fixed on host)

Device pipeline per core:
  Phase Q: q-table slice for own node range: QT[r*UROWS + n] = featT_n @ AqX
  Phase A: per 128-edge tile (cut at dst boundaries, dst-sorted):
      gather feat[src] rows + QT[dst] rows (indirect DMA, batched),
      transpose feat rows on PE, k/v via matmul, fused dot+bias -> exp,
      one-hot segment matmul -> per-tile node partial rows in PSUM,
      indirect-scatter rows (globally unique) into table UT.
  Phase B: upd = U0/S0 + U1/S1 per 128-node block, transpose, @Wo + bo'.
"""
import sys

sys.path.insert(0, "/opt/trn_rl_repo")
import numpy as np

N = 200000
D = 128
H = 2
HF = 64
SCALE = HF ** -0.5
NC = 8
NPC = 25000              # nodes per core
NBLK = 196               # 128-row blocks per core (25088 rows)
UROWS = NBLK * 128       # 25088 padded node rows per relation
TROWS = 2 * UROWS + 128  # table rows (rel0 | rel1 | scratch)
DUMMY = 2 * UROWS        # scratch row for padded scatter slots
TT = 540                 # edge tiles per relation per core (max ~500 used)
G = 30                   # tiles per DMA chunk
GRP = 3                  # tiles fused per compute group
NCHUNK = 2 * TT // G     # 36 chunks (first 18 = rel0, last 18 = rel1)
CPR = TT // G            # chunks per relation
GQ = 14                  # phase-Q blocks per DMA chunk
BB = 7                   # phase-B blocks per DMA chunk
DT_W = None              # weight/matmul dtype knob (None -> float32)


def _fold_weights(inp):
    f8 = lambda x: np.asarray(x, np.float64)
    Wk, bk = f8(inp["Wk"]), f8(inp["bk"])
    Wq, bq = f8(inp["Wq"]), f8(inp["bq"])
    Wv, bv = f8(inp["Wv"]), f8(inp["bv"])
    Wo, bo = f8(inp["Wo"]), f8(inp["bo"])
    out = {}
    cv_sum = np.zeros(D)
    wvs = []
    for r in range(2):
        We, be = f8(inp[f"We{r}"]), f8(inp[f"be{r}"])
        Ak = Wk @ We
        ck = bk @ We + be
        Aq = (Wq @ We) * SCALE
        cq = (bq @ We + be) * SCALE
        Av = Wv @ We
        cv = bv @ We + be
        m0 = np.zeros(D)
        m0[:HF] = 1.0
        m1 = np.zeros(D)
        m1[HF:] = 1.0
        kb0, kb1 = Ak @ (cq * m0), Ak @ (cq * m1)
        qb0, qb1 = Aq @ (ck * m0), Aq @ (ck * m1)
        out[f"AkX{r}"] = np.concatenate([Ak, kb0[:, None], kb1[:, None]], 1).astype(np.float32)
        out[f"AqX{r}"] = np.concatenate([Aq, qb0[:, None], qb1[:, None]], 1).astype(np.float32)
        out[f"Av{r}"] = Av.astype(np.float32)
        cv_sum += cv
        wvs.append((cv @ Wo).astype(np.float32))  # host fix rows for edgeless nodes
    out["Wo"] = np.asarray(inp["Wo"], np.float32)
    bo_eff = bo + cv_sum @ Wo
    out["boB"] = np.broadcast_to(bo_eff.astype(np.float32), (128, D)).copy()
    out["wv0"], out["wv1"] = wvs
    return out


def _tile_edges(src, dst, lo):
    """Partition this core's edges of one relation into dst-boundary-aligned
    tiles of <=128 edges. Returns per-tile arrays [TT,128]:
    src ids, dst rows (0-based in range), relrank (f32, -1 pad), outrow
    (0-based node row, -1 = unassigned spare slot)."""
    sel = (dst >= lo) & (dst < lo + NPC)
    es = src[sel]
    ed = dst[sel] - lo
    o = np.argsort(ed, kind="stable")
    es, ed = es[o], ed[o]
    E = len(es)
    a_src = np.zeros((TT, 128), np.int32)
    a_drow = np.zeros((TT, 128), np.int32)
    a_rr = np.full((TT, 128), -1.0, np.float32)
    a_or = np.full((TT, 128), -1, np.int64)
    covered = []
    start = 0
    t = 0
    while start < E:
        end = min(start + 128, E)
        if end < E:
            end = int(np.searchsorted(ed, ed[end], side="left"))
            assert end > start, "node degree > 128 unsupported"
        n = end - start
        uniq = np.unique(ed[start:end])
        W = len(uniq)
        assert t < TT, "TT too small"
        a_src[t, :n] = es[start:end]
        a_drow[t, :n] = ed[start:end]
        a_rr[t, :n] = np.searchsorted(uniq, ed[start:end]).astype(np.float32)
        a_or[t, :W] = uniq
        covered.append(uniq)
        start = end
        t += 1
    # fill spare outrow slots with rows never covered (edgeless + 25000..25087)
    missing = np.setdiff1d(np.arange(UROWS), np.concatenate(covered) if covered else [])
    mpos = 0
    for tt in range(TT):
        free = np.nonzero(a_or[tt] < 0)[0]
        if mpos >= len(missing):
            a_or[tt, free] = DUMMY_LOCAL
            continue
        take = min(len(free), len(missing) - mpos)
        a_or[tt, free[:take]] = missing[mpos : mpos + take]
        a_or[tt, free[take:]] = DUMMY_LOCAL
        mpos += take
    assert mpos >= len(missing), "not enough spare slots for missing rows"
    return a_src, a_drow, a_rr, a_or


DUMMY_LOCAL = -12345  # placeholder replaced with DUMMY after rel offset applied


def _prep_core(i, inp):
    lo = i * NPC
    maps = {}
    srcs, drows, rrs, ors = [], [], [], []
    for r in range(2):
        a_src, a_drow, a_rr, a_or = _tile_edges(inp[f"src{r}"], inp[f"dst{r}"], lo)
        qrow = a_drow.astype(np.int64) + r * UROWS
        orow = a_or.copy()
        real = orow != DUMMY_LOCAL
        orow[real] += r * UROWS
        orow[~real] = DUMMY
        srcs.append(a_src)
        drows.append(qrow.astype(np.int32))
        rrs.append(a_rr)
        ors.append(orow.astype(np.int32))

    def chunked(arr2):  # [2*TT,128] -> [NCHUNK,128,G]
        a = np.concatenate(arr2, 0)
        return np.ascontiguousarray(
            a.reshape(NCHUNK, G, 128).transpose(0, 2, 1)
        )

    maps["a_src"] = chunked(srcs)
    maps["a_qrow"] = chunked(drows)
    maps["a_rr"] = chunked(rrs)
    maps["a_or"] = chunked(ors)
    fs = np.zeros((UROWS, D), np.float32)
    fs[:NPC] = inp["feat"][lo : lo + NPC]
    maps["fsliceT"] = np.ascontiguousarray(fs.T)  # [128, UROWS]
    return maps


_NC_CACHE = {}
TRACE = False
DEBUG = False
LAST = {}


def _build():
    import concourse.bass as bass
    import concourse.mybir as mybir
    import concourse.tile as tile
    from concourse.bacc import Bacc

    f32 = mybir.dt.float32
    i32 = mybir.dt.int32
    C = D + 2  # 130
    nc = Bacc()
    P = {}
    feat = nc.declare_dram_parameter("feat", [N, D], f32, isOutput=False)
    fsliceT = nc.declare_dram_parameter("fsliceT", [D, UROWS], f32, isOutput=False)
    a_src = nc.declare_dram_parameter("a_src", [NCHUNK, 128, G], i32, isOutput=False)
    a_qrow = nc.declare_dram_parameter("a_qrow", [NCHUNK, 128, G], i32, isOutput=False)
    a_rr = nc.declare_dram_parameter("a_rr", [NCHUNK, 128, G], f32, isOutput=False)
    a_or = nc.declare_dram_parameter("a_or", [NCHUNK, 128, G], i32, isOutput=False)
    wts = {}
    for nm in ("AkX0", "AkX1", "AqX0", "AqX1"):
        wts[nm] = nc.declare_dram_parameter(nm, [D, D + 2], f32, isOutput=False)
    for nm in ("Av0", "Av1", "Wo", "boB"):
        wts[nm] = nc.declare_dram_parameter(nm, [D, D], f32, isOutput=False)
    iota = nc.declare_dram_parameter("iota", [128, 128], f32, isOutput=False)
    ident = nc.declare_dram_parameter("ident", [128, 128], f32, isOutput=False)
    out = nc.declare_dram_parameter("out", [UROWS, D], f32, isOutput=True)
    dbg = {}
    if DEBUG:
        dbg["fs0"] = nc.declare_dram_parameter("dbg_fs0", [128, G * 128], f32, isOutput=True)
        dbg["qg0"] = nc.declare_dram_parameter("dbg_qg0", [128, G * C], f32, isOutput=True)
        dbg["uc0"] = nc.declare_dram_parameter("dbg_uc0", [128, G * C], f32, isOutput=True)
        dbg["qt0"] = nc.declare_dram_parameter("dbg_qt0", [128, 2 * C], f32, isOutput=True)
        dbg["u0c"] = nc.declare_dram_parameter("dbg_u0c", [128, BB * C], f32, isOutput=True)
        dbg["ob0"] = nc.declare_dram_parameter("dbg_ob0", [128, BB * 128], f32, isOutput=True)
    QT = nc.dram_tensor("QT", [TROWS, D + 2], f32)
    UT = nc.dram_tensor("UT", [TROWS, D + 2], f32)

    with tile.TileContext(nc) as tc:
        with tc.tile_pool(name="cst", bufs=1) as cst:
            w_sb = {}
            for nm, hnd in wts.items():
                t = cst.tile(list(hnd.shape), f32, tag=nm)
                nc.sync.dma_start(out=t[:], in_=hnd[:, :])
                w_sb[nm] = t
            iota_t = cst.tile([128, 128], f32, tag="iota")
            nc.sync.dma_start(out=iota_t[:], in_=iota[:, :])
            ident_t = cst.tile([128, 128], f32, tag="ident")
            nc.sync.dma_start(out=ident_t[:], in_=ident[:, :])

            # ---------------- Phase Q: build q-table for own range ----------
            with (
                tc.tile_pool(name="qsb", bufs=2) as qsb,
                tc.tile_pool(name="qps", bufs=2, space="PSUM") as qps,
            ):
                for r in range(2):
                    for cqk in range(NBLK // GQ):
                        fT = qsb.tile([128, GQ * 128], f32, tag="fT")
                        nc.sync.dma_start(
                            out=fT[:],
                            in_=fsliceT[:, cqk * GQ * 128 : (cqk + 1) * GQ * 128],
                        )
                        qbuf = qsb.tile([128, GQ * C], f32, tag="qbuf")
                        for j in range(GQ):
                            q_ps = qps.tile([128, C], f32, tag="q_ps")
                            nc.tensor.matmul(
                                out=q_ps[:],
                                lhsT=fT[:, j * 128 : (j + 1) * 128],
                                rhs=w_sb[f"AqX{r}"][:],
                                start=True,
                                stop=True,
                            )
                            nc.scalar.activation(
                                out=qbuf[:, j * C : (j + 1) * C],
                                in_=q_ps[:],
                                func=mybir.ActivationFunctionType.Copy,
                            )
                        lo = r * UROWS + cqk * GQ * 128
                        dst_ap = QT[lo : lo + GQ * 128, :].rearrange(
                            "(b p) c -> p b c", p=128
                        )
                        nc.sync.dma_start(out=dst_ap, in_=qbuf[:])

            # ---------------- Phase A: edge tiles ---------------------------
            with (
                tc.tile_pool(name="asb", bufs=2) as asb,
                tc.tile_pool(name="aps", bufs=2, space="PSUM") as aps,
            ):
                if DEBUG:
                    qt_t = asb.tile([128, 2 * C], f32, tag="qt_t")
                    nc.sync.dma_start(out=qt_t[:, 0:C], in_=QT[0:128, :])
                    nc.sync.dma_start(out=qt_t[:, C : 2 * C], in_=QT[UROWS : UROWS + 128, :])
                    nc.sync.dma_start(out=dbg["qt0"][:, :], in_=qt_t[:])
                for c in range(NCHUNK):
                    r = c // CPR
                    src_t = asb.tile([128, G], i32, tag="src_t")
                    qrow_t = asb.tile([128, G], i32, tag="qrow_t")
                    rr_t = asb.tile([128, G], f32, tag="rr_t")
                    or_t = asb.tile([128, G], i32, tag="or_t")
                    nc.sync.dma_start(out=src_t[:], in_=a_src[c, :, :])
                    nc.sync.dma_start(out=qrow_t[:], in_=a_qrow[c, :, :])
                    nc.sync.dma_start(out=rr_t[:], in_=a_rr[c, :, :])
                    nc.sync.dma_start(out=or_t[:], in_=a_or[c, :, :])

                    fs_raw = asb.tile([128, G * 128], f32, tag="fs_raw")
                    q_g = asb.tile([128, G * C], f32, tag="q_g")
                    for t in range(G):
                        nc.gpsimd.indirect_dma_start(
                            out=fs_raw[:, t * 128 : (t + 1) * 128],
                            out_offset=None,
                            in_=feat[:, :],
                            in_offset=bass.IndirectOffsetOnAxis(
                                ap=src_t[:, t : t + 1], axis=0
                            ),
                        )
                        nc.gpsimd.indirect_dma_start(
                            out=q_g[:, t * C : (t + 1) * C],
                            out_offset=None,
                            in_=QT[:, :],
                            in_offset=bass.IndirectOffsetOnAxis(
                                ap=qrow_t[:, t : t + 1], axis=0
                            ),
                        )
                    u_chunk = asb.tile([128, G * C], f32, tag="u_chunk")

                    for grp in range(G // GRP):
                        b0 = grp * GRP
                        fsT_ps = aps.tile([128, GRP * 128], f32, tag="fsT_ps")
                        for t in range(GRP):
                            nc.tensor.transpose(
                                out=fsT_ps[:, t * 128 : (t + 1) * 128],
                                in_=fs_raw[:, (b0 + t) * 128 : (b0 + t + 1) * 128],
                                identity=ident_t[:],
                            )
                        fsT = asb.tile([128, GRP * 128], f32, tag="fsT")
                        nc.scalar.activation(
                            out=fsT[:], in_=fsT_ps[:],
                            func=mybir.ActivationFunctionType.Copy,
                        )
                        k_ps = aps.tile([128, GRP * C], f32, tag="k_ps")
                        v_ps = aps.tile([128, GRP * 128], f32, tag="v_ps")
                        for t in range(GRP):
                            nc.tensor.matmul(
                                out=k_ps[:, t * C : (t + 1) * C],
                                lhsT=fsT[:, t * 128 : (t + 1) * 128],
                                rhs=w_sb[f"AkX{r}"][:],
                                start=True, stop=True,
                            )
                            nc.tensor.matmul(
                                out=v_ps[:, t * 128 : (t + 1) * 128],
                                lhsT=fsT[:, t * 128 : (t + 1) * 128],
                                rhs=w_sb[f"Av{r}"][:],
                                start=True, stop=True,
                            )
                        k3 = k_ps[:].rearrange("p (t c) -> p t c", t=GRP)
                        q3 = q_g[:, b0 * C : (b0 + GRP) * C].rearrange(
                            "p (t c) -> p t c", t=GRP
                        )
                        # bias_t[p,(t,h)] = k_ext + q_ext
                        bias_t = asb.tile([128, GRP * H], f32, tag="bias_t")
                        nc.vector.tensor_add(
                            out=bias_t[:].rearrange("p (t h) -> p t h", t=GRP),
                            in0=k3[:, :, 128:130],
                            in1=q3[:, :, 128:130],
                        )
                        kq = asb.tile([128, GRP * 128], f32, tag="kq")
                        nc.vector.tensor_mul(
                            out=kq[:].rearrange("p (t c) -> p t c", t=GRP),
                            in0=k3[:, :, 0:128],
                            in1=q3[:, :, 0:128],
                        )
                        logit = asb.tile([128, GRP * H], f32, tag="logit")
                        nc.vector.reduce_sum(
                            out=logit[:].rearrange("p (th one) -> p th one", one=1),
                            in_=kq[:].rearrange("p (th c) -> p th c", c=HF),
                            axis=mybir.AxisListType.X,
                        )
                        nc.vector.tensor_add(out=logit[:], in0=logit[:], in1=bias_t[:])
                        p3 = asb.tile([128, GRP * H], f32, tag="p3")
                        nc.scalar.activation(
                            out=p3[:], in_=logit[:],
                            func=mybir.ActivationFunctionType.Exp,
                        )
                        oh = asb.tile([128, GRP * 128], f32, tag="oh")
                        for t in range(GRP):
                            nc.vector.tensor_scalar(
                                out=oh[:, t * 128 : (t + 1) * 128],
                                in0=iota_t[:],
                                scalar1=rr_t[:, b0 + t : b0 + t + 1],
                                scalar2=None,
                                op0=mybir.AluOpType.is_equal,
                            )
                        msg = asb.tile([128, GRP * C], f32, tag="msg")
                        m4 = msg[:].rearrange("p (t c) -> p t c", t=GRP)[
                            :, :, 0:128
                        ].rearrange("p t (h c) -> p t h c", h=H)
                        v4 = v_ps[:].rearrange("p (t h c) -> p t h c", t=GRP, h=H)
                        p4 = p3[:].rearrange("p (t h) -> p t h", t=GRP)
                        nc.vector.tensor_mul(
                            out=m4[:],
                            in0=v4[:],
                            in1=p4.to_broadcast([128, GRP, H, HF]),
                        )
                        nc.vector.tensor_copy(
                            out=msg[:].rearrange("p (t c) -> p t c", t=GRP)[
                                :, :, 128:130
                            ],
                            in_=p4[:],
                        )
                        u_ps = aps.tile([128, GRP * C], f32, tag="u_ps")
                        for t in range(GRP):
                            nc.tensor.matmul(
                                out=u_ps[:, t * C : (t + 1) * C],
                                lhsT=oh[:, t * 128 : (t + 1) * 128],
                                rhs=msg[:, t * C : (t + 1) * C],
                                start=True, stop=True,
                            )
                        nc.scalar.activation(
                            out=u_chunk[:, b0 * C : (b0 + GRP) * C],
                            in_=u_ps[:],
                            func=mybir.ActivationFunctionType.Copy,
                        )
                    if DEBUG and c == 0:
                        nc.sync.dma_start(out=dbg["fs0"][:, :], in_=fs_raw[:])
                        nc.sync.dma_start(out=dbg["qg0"][:, :], in_=q_g[:])
                        nc.sync.dma_start(out=dbg["uc0"][:, :], in_=u_chunk[:])
                    for t in range(G):
                        nc.gpsimd.indirect_dma_start(
                            out=UT[:, :],
                            out_offset=bass.IndirectOffsetOnAxis(
                                ap=or_t[:, t : t + 1], axis=0
                            ),
                            in_=u_chunk[:, t * C : (t + 1) * C],
                            in_offset=None,
                        )

            # ---------------- Phase B: normalize + out transform ------------
            with (
                tc.tile_pool(name="bsb", bufs=2) as bsb,
                tc.tile_pool(name="bps", bufs=2, space="PSUM") as bps,
            ):
                for cb in range(NBLK // BB):
                    u0c = bsb.tile([128, BB * C], f32, tag="u0c")
                    u1c = bsb.tile([128, BB * C], f32, tag="u1c")
                    lo0 = cb * BB * 128
                    nc.sync.dma_start(
                        out=u0c[:],
                        in_=UT[lo0 : lo0 + BB * 128, :].rearrange(
                            "(b p) c -> p b c", p=128
                        ),
                    )
                    nc.sync.dma_start(
                        out=u1c[:],
                        in_=UT[UROWS + lo0 : UROWS + lo0 + BB * 128, :].rearrange(
                            "(b p) c -> p b c", p=128
                        ),
                    )
                    obuf = bsb.tile([128, BB * 128], f32, tag="obuf")
                    for j in range(BB):
                        sm = bsb.tile([128, 2 * H], f32, tag="sm")
                        nc.vector.tensor_scalar(
                            out=sm[:, 0:2], in0=u0c[:, j * C + 128 : j * C + 130],
                            scalar1=0.5, scalar2=None, op0=mybir.AluOpType.max,
                        )
                        nc.vector.tensor_scalar(
                            out=sm[:, 2:4], in0=u1c[:, j * C + 128 : j * C + 130],
                            scalar1=0.5, scalar2=None, op0=mybir.AluOpType.max,
                        )
                        rcp = bsb.tile([128, 2 * H], f32, tag="rcp")
                        nc.vector.reciprocal(out=rcp[:], in_=sm[:])
                        upd = bsb.tile([128, 128], f32, tag="upd")
                        t1 = bsb.tile([128, 128], f32, tag="t1")
                        for h in range(H):
                            nc.scalar.activation(
                                out=upd[:, h * HF : (h + 1) * HF],
                                in_=u0c[:, j * C + h * HF : j * C + (h + 1) * HF],
                                func=mybir.ActivationFunctionType.Copy,
                                scale=rcp[:, h : h + 1],
                            )
                            nc.scalar.activation(
                                out=t1[:, h * HF : (h + 1) * HF],
                                in_=u1c[:, j * C + h * HF : j * C + (h + 1) * HF],
                                func=mybir.ActivationFunctionType.Copy,
                                scale=rcp[:, 2 + h : 3 + h],
                            )
                        nc.vector.tensor_add(out=upd[:], in0=upd[:], in1=t1[:])
                        uT_ps = bps.tile([128, 128], f32, tag="uT_ps")
                        nc.tensor.transpose(
                            out=uT_ps[:], in_=upd[:], identity=ident_t[:]
                        )
                        uT = bsb.tile([128, 128], f32, tag="uT")
                        nc.vector.tensor_copy(out=uT[:], in_=uT_ps[:])
                        o_ps = bps.tile([128, 128], f32, tag="o_ps")
                        nc.tensor.matmul(
                            out=o_ps[:], lhsT=uT[:], rhs=w_sb["Wo"][:],
                            start=True, stop=True,
                        )
                        nc.vector.tensor_add(
                            out=obuf[:, j * 128 : (j + 1) * 128],
                            in0=o_ps[:],
                            in1=w_sb["boB"][:],
                        )
                    if DEBUG and cb == 0:
                        nc.sync.dma_start(out=dbg["u0c"][:, :], in_=u0c[:])
                        nc.sync.dma_start(out=dbg["ob0"][:, :], in_=obuf[:])
                    nc.sync.dma_start(
                        out=out[lo0 : lo0 + BB * 128, :].rearrange(
                            "(b p) c -> p b c", p=128
                        ),
                        in_=obuf[:],
                    )
    nc.finalize()
    return nc


def _get_nc():
    if "nc" not in _NC_CACHE:
        _NC_CACHE["nc"] = _build()
    return _NC_CACHE["nc"]


def kernel(**inputs):
    from concourse.bass_utils import run_bass_kernel_spmd

    w = _fold_weights(inputs)
    feat = np.ascontiguousarray(inputs["feat"], np.float32)
    shared = {
        "feat": feat,
        "AkX0": w["AkX0"], "AkX1": w["AkX1"],
        "AqX0": w["AqX0"], "AqX1": w["AqX1"],
        "Av0": w["Av0"], "Av1": w["Av1"],
        "Wo": w["Wo"], "boB": w["boB"],
        "iota": np.broadcast_to(
            np.arange(128, dtype=np.float32), (128, 128)
        ).copy(),
        "ident": np.eye(128, dtype=np.float32),
    }
    in_maps = []
    for i in range(NC):
        m = dict(shared)
        m.update(_prep_core(i, inputs))
        in_maps.append(m)

    nc = _get_nc()
    try:
        res = run_bass_kernel_spmd(nc, in_maps, list(range(NC)), trace=TRACE)
    except ModuleNotFoundError:
        res = run_bass_kernel_spmd(nc, in_maps, list(range(NC)))
    LAST["res"] = res
    out = np.concatenate(
        [res.results[i]["out"][:NPC] for i in range(NC)], axis=0
    )
    # host fix-up: nodes with no incoming edges in relation r lack that
    # relation's folded value-bias contribution
    for r, wv in ((0, w["wv0"]), (1, w["wv1"])):
        deg = np.bincount(inputs[f"dst{r}"], minlength=N)
        out[deg == 0] -= wv
    return out.astype(np.float32)


if __name__ == "__main__":
    import reference

    inp = {k: np.asarray(v) for k, v in reference.setup_inputs().items()}
    got = kernel(**inp)
    exp = np.asarray(reference.reference(**inp))
    err = np.abs(got - exp).max()
    rel = err / np.abs(exp).max()
    print("absmax err:", err, "rel:", rel)


# revision 13
# speedup vs baseline: 1.4052x; 1.4052x over previous
#!/usr/bin/env python3
"""
Trainium2 Bass kernel for nn_AttentionLayer (2-relation GNN edge-softmax
attention message passing), 8 NeuronCores, SPMD.

Sharding: nodes are split into 8 contiguous ranges of 25000 (dst-based edge
parallelism). Core i processes all edges (both relations) whose destination
falls in its range, so per-destination segment softmax/sums complete locally
with no cross-core reduction. Output rows are produced per-core and
concatenated on the host.

Per relation r the math is refactored as
    k = feat @ Ak + ck,  q = feat @ Aq + cq  (Aq, cq pre-scaled by HF^-0.5)
    v = feat @ Av + cv           with A* = W* @ We_r etc. (host-folded)
    logit_e,h = <(k[src]+ck), (q[dst]+cq)>_h
    p = exp(logit)               (no max subtraction: |logit| < ~0.01)
    upd = segsum(p * v[src]) / segsum(p) + cv  (cv folded into bias;
                                               edgeless nodes fixed on host)

Device pipeline per core:
  Phase Q: q-table slice for own node range: QT[r*UROWS + n] = featT_n @ AqX
  Phase A: per 128-edge tile (cut at dst boundaries, dst-sorted):
      gather feat[src] rows + QT[dst] rows (indirect DMA, batched),
      transpose feat rows on PE, k/v via matmul, fused dot+bias -> exp,
      one-hot segment matmul -> per-tile node partial rows in PSUM,
      indirect-scatter rows (globally unique) into table UT.
  Phase B: upd = U0/S0 + U1/S1 per 128-node block, transpose, @Wo + bo'.
"""
import sys

sys.path.insert(0, "/opt/trn_rl_repo")
import numpy as np

N = 200000
D = 128
H = 2
HF = 64
SCALE = HF ** -0.5
NC = 8
NPC = 25000              # nodes per core
NBLK = 196               # 128-row blocks per core (25088 rows)
UROWS = NBLK * 128       # 25088 padded node rows per relation
TROWS = 2 * UROWS + 128  # table rows (rel0 | rel1 | scratch)
DUMMY = 2 * UROWS        # scratch row for padded scatter slots
TT = 540                 # edge tiles per relation per core (max ~500 used)
G = 30                   # tiles per DMA chunk
GRP = 3                  # tiles fused per compute group
NCHUNK = 2 * TT // G     # 36 chunks (first 18 = rel0, last 18 = rel1)
CPR = TT // G            # chunks per relation
GQ = 14                  # phase-Q blocks per DMA chunk
BB = 7                   # phase-B blocks per DMA chunk
DT_W = None              # weight/matmul dtype knob (None -> float32)


def _fold_weights(inp):
    f8 = lambda x: np.asarray(x, np.float64)
    Wk, bk = f8(inp["Wk"]), f8(inp["bk"])
    Wq, bq = f8(inp["Wq"]), f8(inp["bq"])
    Wv, bv = f8(inp["Wv"]), f8(inp["bv"])
    Wo, bo = f8(inp["Wo"]), f8(inp["bo"])
    out = {}
    cv_sum = np.zeros(D)
    wvs = []
    for r in range(2):
        We, be = f8(inp[f"We{r}"]), f8(inp[f"be{r}"])
        Ak = Wk @ We
        ck = bk @ We + be
        Aq = (Wq @ We) * SCALE
        cq = (bq @ We + be) * SCALE
        Av = Wv @ We
        cv = bv @ We + be
        m0 = np.zeros(D)
        m0[:HF] = 1.0
        m1 = np.zeros(D)
        m1[HF:] = 1.0
        kb0, kb1 = Ak @ (cq * m0), Ak @ (cq * m1)
        qb0, qb1 = Aq @ (ck * m0), Aq @ (ck * m1)
        out[f"AkX{r}"] = np.concatenate([Ak, kb0[:, None], kb1[:, None]], 1).astype(np.float32)
        out[f"AqX{r}"] = np.concatenate([Aq, qb0[:, None], qb1[:, None]], 1).astype(np.float32)
        out[f"Av{r}"] = Av.astype(np.float32)
        cv_sum += cv
        wvs.append((cv @ Wo).astype(np.float32))  # host fix rows for edgeless nodes
    out["Wo"] = np.asarray(inp["Wo"], np.float32)
    bo_eff = bo + cv_sum @ Wo
    out["boB"] = np.broadcast_to(bo_eff.astype(np.float32), (128, D)).copy()
    out["wv0"], out["wv1"] = wvs
    return out


def _tile_edges(src, dst, lo):
    """Partition this core's edges of one relation into dst-boundary-aligned
    tiles of <=128 edges. Returns per-tile arrays [TT,128]:
    src ids, dst rows (0-based in range), relrank (f32, -1 pad), outrow
    (0-based node row, -1 = unassigned spare slot)."""
    sel = (dst >= lo) & (dst < lo + NPC)
    es = src[sel]
    ed = dst[sel] - lo
    o = np.argsort(ed, kind="stable")
    es, ed = es[o], ed[o]
    E = len(es)
    a_src = np.zeros((TT, 128), np.int32)
    a_drow = np.zeros((TT, 128), np.int32)
    a_rr = np.full((TT, 128), -1.0, np.float32)
    a_or = np.full((TT, 128), -1, np.int64)
    covered = []
    start = 0
    t = 0
    while start < E:
        end = min(start + 128, E)
        if end < E:
            end = int(np.searchsorted(ed, ed[end], side="left"))
            assert end > start, "node degree > 128 unsupported"
        n = end - start
        uniq = np.unique(ed[start:end])
        W = len(uniq)
        assert t < TT, "TT too small"
        a_src[t, :n] = es[start:end]
        a_drow[t, :n] = ed[start:end]
        a_rr[t, :n] = np.searchsorted(uniq, ed[start:end]).astype(np.float32)
        a_or[t, :W] = uniq
        covered.append(uniq)
        start = end
        t += 1
    # fill spare outrow slots with rows never covered (edgeless + 25000..25087)
    missing = np.setdiff1d(np.arange(UROWS), np.concatenate(covered) if covered else [])
    mpos = 0
    for tt in range(TT):
        free = np.nonzero(a_or[tt] < 0)[0]
        if mpos >= len(missing):
            a_or[tt, free] = DUMMY_LOCAL
            continue
        take = min(len(free), len(missing) - mpos)
        a_or[tt, free[:take]] = missing[mpos : mpos + take]
        a_or[tt, free[take:]] = DUMMY_LOCAL
        mpos += take
    assert mpos >= len(missing), "not enough spare slots for missing rows"
    return a_src, a_drow, a_rr, a_or


DUMMY_LOCAL = -12345  # placeholder replaced with DUMMY after rel offset applied


def _prep_core(i, inp):
    lo = i * NPC
    maps = {}
    srcs, drows, rrs, cns = [], [], [], []
    for r in range(2):
        a_src, a_drow, a_rr, a_or = _tile_edges(inp[f"src{r}"], inp[f"dst{r}"], lo)
        qrow = a_drow.astype(np.int64) + r * UROWS
        srcs.append(a_src)
        drows.append(qrow.astype(np.int32))
        rrs.append(a_rr)
        # compact position of each node row: every row (incl. edgeless) sits in
        # exactly one (tile, window-slot); padding slots produce zero rows
        cn = np.empty(UROWS, np.int64)
        m = a_or != DUMMY_LOCAL
        pos = np.arange(TT * 128).reshape(TT, 128)
        cn[a_or[m]] = pos[m] + r * TT * 128
        cns.append(cn)

    def chunked(arr2):  # [2*TT,128] -> [NCHUNK,128,G]
        a = np.concatenate(arr2, 0)
        return np.ascontiguousarray(
            a.reshape(NCHUNK, G, 128).transpose(0, 2, 1)
        )

    maps["a_src"] = chunked(srcs)
    maps["a_qrow"] = chunked(drows)
    maps["a_rr"] = chunked(rrs)
    for r in range(2):
        bc = cns[r].reshape(NBLK, 128).astype(np.int32)      # [block, p]
        maps[f"a_bc{r}"] = np.ascontiguousarray(
            bc.reshape(NBLK // BB, BB, 128).transpose(0, 2, 1)
        )  # [28, 128, BB]
    fs = np.zeros((UROWS, D), np.float32)
    fs[:NPC] = inp["feat"][lo : lo + NPC]
    maps["fsliceT"] = np.ascontiguousarray(fs.T)  # [128, UROWS]
    return maps


_NC_CACHE = {}
TRACE = False
DEBUG = False
LAST = {}


def _build():
    import concourse.bass as bass
    import concourse.mybir as mybir
    import concourse.tile as tile
    from concourse.bacc import Bacc

    f32 = mybir.dt.float32
    i32 = mybir.dt.int32
    C = D + 2  # 130
    nc = Bacc()
    P = {}
    feat = nc.declare_dram_parameter("feat", [N, D], f32, isOutput=False)
    fsliceT = nc.declare_dram_parameter("fsliceT", [D, UROWS], f32, isOutput=False)
    a_src = nc.declare_dram_parameter("a_src", [NCHUNK, 128, G], i32, isOutput=False)
    a_qrow = nc.declare_dram_parameter("a_qrow", [NCHUNK, 128, G], i32, isOutput=False)
    a_rr = nc.declare_dram_parameter("a_rr", [NCHUNK, 128, G], f32, isOutput=False)
    a_bc0 = nc.declare_dram_parameter("a_bc0", [NBLK // BB, 128, BB], i32, isOutput=False)
    a_bc1 = nc.declare_dram_parameter("a_bc1", [NBLK // BB, 128, BB], i32, isOutput=False)
    wts = {}
    for nm in ("AkX0", "AkX1", "AqX0", "AqX1"):
        wts[nm] = nc.declare_dram_parameter(nm, [D, D + 2], f32, isOutput=False)
    for nm in ("Av0", "Av1", "Wo", "boB"):
        wts[nm] = nc.declare_dram_parameter(nm, [D, D], f32, isOutput=False)
    iota = nc.declare_dram_parameter("iota", [128, 128], f32, isOutput=False)
    ident = nc.declare_dram_parameter("ident", [128, 128], f32, isOutput=False)
    out = nc.declare_dram_parameter("out", [UROWS, D], f32, isOutput=True)
    dbg = {}
    if DEBUG:
        dbg["fs0"] = nc.declare_dram_parameter("dbg_fs0", [128, G * 128], f32, isOutput=True)
        dbg["qg0"] = nc.declare_dram_parameter("dbg_qg0", [128, G * C], f32, isOutput=True)
        dbg["uc0"] = nc.declare_dram_parameter("dbg_uc0", [128, G * C], f32, isOutput=True)
        dbg["qt0"] = nc.declare_dram_parameter("dbg_qt0", [128, 2 * C], f32, isOutput=True)
        dbg["u0c"] = nc.declare_dram_parameter("dbg_u0c", [128, BB * C], f32, isOutput=True)
        dbg["ob0"] = nc.declare_dram_parameter("dbg_ob0", [128, BB * 128], f32, isOutput=True)
    QT = nc.dram_tensor("QT", [TROWS, D + 2], f32)
    UC = nc.dram_tensor("UC", [2 * TT * 128, D + 2], f32)

    with tile.TileContext(nc) as tc:
        with tc.tile_pool(name="cst", bufs=1) as cst:
            w_sb = {}
            for nm, hnd in wts.items():
                t = cst.tile(list(hnd.shape), f32, tag=nm)
                nc.sync.dma_start(out=t[:], in_=hnd[:, :])
                w_sb[nm] = t
            iota_t = cst.tile([128, 128], f32, tag="iota")
            nc.sync.dma_start(out=iota_t[:], in_=iota[:, :])
            ident_t = cst.tile([128, 128], f32, tag="ident")
            nc.sync.dma_start(out=ident_t[:], in_=ident[:, :])

            # ---------------- Phase Q: build q-table for own range ----------
            with (
                tc.tile_pool(name="qsb", bufs=2) as qsb,
                tc.tile_pool(name="qps", bufs=2, space="PSUM") as qps,
            ):
                for r in range(2):
                    for cqk in range(NBLK // GQ):
                        fT = qsb.tile([128, GQ * 128], f32, tag="fT")
                        nc.sync.dma_start(
                            out=fT[:],
                            in_=fsliceT[:, cqk * GQ * 128 : (cqk + 1) * GQ * 128],
                        )
                        qbuf = qsb.tile([128, GQ * C], f32, tag="qbuf")
                        for j in range(GQ):
                            q_ps = qps.tile([128, C], f32, tag="q_ps")
                            nc.tensor.matmul(
                                out=q_ps[:],
                                lhsT=fT[:, j * 128 : (j + 1) * 128],
                                rhs=w_sb[f"AqX{r}"][:],
                                start=True,
                                stop=True,
                            )
                            nc.scalar.activation(
                                out=qbuf[:, j * C : (j + 1) * C],
                                in_=q_ps[:],
                                func=mybir.ActivationFunctionType.Copy,
                            )
                        lo = r * UROWS + cqk * GQ * 128
                        dst_ap = QT[lo : lo + GQ * 128, :].rearrange(
                            "(b p) c -> p b c", p=128
                        )
                        nc.sync.dma_start(out=dst_ap, in_=qbuf[:])

            # ---------------- Phase A: edge tiles ---------------------------
            with (
                tc.tile_pool(name="asb", bufs=2) as asb,
                tc.tile_pool(name="aps", bufs=2, space="PSUM") as aps,
            ):
                if DEBUG:
                    qt_t = asb.tile([128, 2 * C], f32, tag="qt_t")
                    nc.sync.dma_start(out=qt_t[:, 0:C], in_=QT[0:128, :])
                    nc.sync.dma_start(out=qt_t[:, C : 2 * C], in_=QT[UROWS : UROWS + 128, :])
                    nc.sync.dma_start(out=dbg["qt0"][:, :], in_=qt_t[:])
                for c in range(NCHUNK):
                    r = c // CPR
                    src_t = asb.tile([128, G], i32, tag="src_t")
                    qrow_t = asb.tile([128, G], i32, tag="qrow_t")
                    rr_t = asb.tile([128, G], f32, tag="rr_t")
                    nc.sync.dma_start(out=src_t[:], in_=a_src[c, :, :])
                    nc.sync.dma_start(out=qrow_t[:], in_=a_qrow[c, :, :])
                    nc.sync.dma_start(out=rr_t[:], in_=a_rr[c, :, :])

                    fs_raw = asb.tile([128, G * 128], f32, tag="fs_raw")
                    q_g = asb.tile([128, G * C], f32, tag="q_g")
                    for t in range(G):
                        nc.gpsimd.indirect_dma_start(
                            out=fs_raw[:, t * 128 : (t + 1) * 128],
                            out_offset=None,
                            in_=feat[:, :],
                            in_offset=bass.IndirectOffsetOnAxis(
                                ap=src_t[:, t : t + 1], axis=0
                            ),
                        )
                        nc.gpsimd.indirect_dma_start(
                            out=q_g[:, t * C : (t + 1) * C],
                            out_offset=None,
                            in_=QT[:, :],
                            in_offset=bass.IndirectOffsetOnAxis(
                                ap=qrow_t[:, t : t + 1], axis=0
                            ),
                        )
                    u_chunk = asb.tile([128, G * C], f32, tag="u_chunk")

                    for grp in range(G // GRP):
                        b0 = grp * GRP
                        fsT_ps = aps.tile([128, GRP * 128], f32, tag="fsT_ps")
                        for t in range(GRP):
                            nc.tensor.transpose(
                                out=fsT_ps[:, t * 128 : (t + 1) * 128],
                                in_=fs_raw[:, (b0 + t) * 128 : (b0 + t + 1) * 128],
                                identity=ident_t[:],
                            )
                        fsT = asb.tile([128, GRP * 128], f32, tag="fsT")
                        nc.scalar.activation(
                            out=fsT[:], in_=fsT_ps[:],
                            func=mybir.ActivationFunctionType.Copy,
                        )
                        k_ps = aps.tile([128, GRP * C], f32, tag="k_ps")
                        v_ps = aps.tile([128, GRP * 128], f32, tag="v_ps")
                        for t in range(GRP):
                            nc.tensor.matmul(
                                out=k_ps[:, t * C : (t + 1) * C],
                                lhsT=fsT[:, t * 128 : (t + 1) * 128],
                                rhs=w_sb[f"AkX{r}"][:],
                                start=True, stop=True,
                            )
                            nc.tensor.matmul(
                                out=v_ps[:, t * 128 : (t + 1) * 128],
                                lhsT=fsT[:, t * 128 : (t + 1) * 128],
                                rhs=w_sb[f"Av{r}"][:],
                                start=True, stop=True,
                            )
                        k3 = k_ps[:].rearrange("p (t c) -> p t c", t=GRP)
                        q3 = q_g[:, b0 * C : (b0 + GRP) * C].rearrange(
                            "p (t c) -> p t c", t=GRP
                        )
                        # bias_t[p,(t,h)] = k_ext + q_ext
                        bias_t = asb.tile([128, GRP * H], f32, tag="bias_t")
                        nc.vector.tensor_add(
                            out=bias_t[:].rearrange("p (t h) -> p t h", t=GRP),
                            in0=k3[:, :, 128:130],
                            in1=q3[:, :, 128:130],
                        )
                        kq = asb.tile([128, GRP * 128], f32, tag="kq")
                        nc.vector.tensor_mul(
                            out=kq[:].rearrange("p (t c) -> p t c", t=GRP),
                            in0=k3[:, :, 0:128],
                            in1=q3[:, :, 0:128],
                        )
                        logit = asb.tile([128, GRP * H], f32, tag="logit")
                        nc.vector.reduce_sum(
                            out=logit[:].rearrange("p (th one) -> p th one", one=1),
                            in_=kq[:].rearrange("p (th c) -> p th c", c=HF),
                            axis=mybir.AxisListType.X,
                        )
                        nc.vector.tensor_add(out=logit[:], in0=logit[:], in1=bias_t[:])
                        p3 = asb.tile([128, GRP * H], f32, tag="p3")
                        nc.scalar.activation(
                            out=p3[:], in_=logit[:],
                            func=mybir.ActivationFunctionType.Exp,
                        )
                        oh = asb.tile([128, GRP * 128], f32, tag="oh")
                        for t in range(GRP):
                            nc.vector.tensor_scalar(
                                out=oh[:, t * 128 : (t + 1) * 128],
                                in0=iota_t[:],
                                scalar1=rr_t[:, b0 + t : b0 + t + 1],
                                scalar2=None,
                                op0=mybir.AluOpType.is_equal,
                            )
                        msg = asb.tile([128, GRP * C], f32, tag="msg")
                        m4 = msg[:].rearrange("p (t c) -> p t c", t=GRP)[
                            :, :, 0:128
                        ].rearrange("p t (h c) -> p t h c", h=H)
                        v4 = v_ps[:].rearrange("p (t h c) -> p t h c", t=GRP, h=H)
                        p4 = p3[:].rearrange("p (t h) -> p t h", t=GRP)
                        nc.vector.tensor_mul(
                            out=m4[:],
                            in0=v4[:],
                            in1=p4.to_broadcast([128, GRP, H, HF]),
                        )
                        nc.vector.tensor_copy(
                            out=msg[:].rearrange("p (t c) -> p t c", t=GRP)[
                                :, :, 128:130
                            ],
                            in_=p4[:],
                        )
                        u_ps = aps.tile([128, GRP * C], f32, tag="u_ps")
                        for t in range(GRP):
                            nc.tensor.matmul(
                                out=u_ps[:, t * C : (t + 1) * C],
                                lhsT=oh[:, t * 128 : (t + 1) * 128],
                                rhs=msg[:, t * C : (t + 1) * C],
                                start=True, stop=True,
                            )
                        nc.scalar.activation(
                            out=u_chunk[:, b0 * C : (b0 + GRP) * C],
                            in_=u_ps[:],
                            func=mybir.ActivationFunctionType.Copy,
                        )
                    if DEBUG and c == 0:
                        nc.sync.dma_start(out=dbg["fs0"][:, :], in_=fs_raw[:])
                        nc.sync.dma_start(out=dbg["qg0"][:, :], in_=q_g[:])
                        nc.sync.dma_start(out=dbg["uc0"][:, :], in_=u_chunk[:])
                    nc.sync.dma_start(
                        out=UC[c * G * 128 : (c + 1) * G * 128, :].rearrange(
                            "(g p) c -> p g c", p=128
                        ),
                        in_=u_chunk[:],
                    )

            # ---------------- Phase B: normalize + out transform ------------
            with (
                tc.tile_pool(name="bsb", bufs=2) as bsb,
                tc.tile_pool(name="bps", bufs=2, space="PSUM") as bps,
            ):
                for cb in range(NBLK // BB):
                    bc0_t = bsb.tile([128, BB], i32, tag="bc0_t")
                    bc1_t = bsb.tile([128, BB], i32, tag="bc1_t")
                    nc.sync.dma_start(out=bc0_t[:], in_=a_bc0[cb, :, :])
                    nc.sync.dma_start(out=bc1_t[:], in_=a_bc1[cb, :, :])
                    u0c = bsb.tile([128, BB * C], f32, tag="u0c")
                    u1c = bsb.tile([128, BB * C], f32, tag="u1c")
                    lo0 = cb * BB * 128
                    for j in range(BB):
                        nc.gpsimd.indirect_dma_start(
                            out=u0c[:, j * C : (j + 1) * C],
                            out_offset=None,
                            in_=UC[:, :],
                            in_offset=bass.IndirectOffsetOnAxis(
                                ap=bc0_t[:, j : j + 1], axis=0
                            ),
                        )
                        nc.gpsimd.indirect_dma_start(
                            out=u1c[:, j * C : (j + 1) * C],
                            out_offset=None,
                            in_=UC[:, :],
                            in_offset=bass.IndirectOffsetOnAxis(
                                ap=bc1_t[:, j : j + 1], axis=0
                            ),
                        )
                    obuf = bsb.tile([128, BB * 128], f32, tag="obuf")
                    for j in range(BB):
                        sm = bsb.tile([128, 2 * H], f32, tag="sm")
                        nc.vector.tensor_scalar(
                            out=sm[:, 0:2], in0=u0c[:, j * C + 128 : j * C + 130],
                            scalar1=0.5, scalar2=None, op0=mybir.AluOpType.max,
                        )
                        nc.vector.tensor_scalar(
                            out=sm[:, 2:4], in0=u1c[:, j * C + 128 : j * C + 130],
                            scalar1=0.5, scalar2=None, op0=mybir.AluOpType.max,
                        )
                        rcp = bsb.tile([128, 2 * H], f32, tag="rcp")
                        nc.vector.reciprocal(out=rcp[:], in_=sm[:])
                        upd = bsb.tile([128, 128], f32, tag="upd")
                        t1 = bsb.tile([128, 128], f32, tag="t1")
                        for h in range(H):
                            nc.scalar.activation(
                                out=upd[:, h * HF : (h + 1) * HF],
                                in_=u0c[:, j * C + h * HF : j * C + (h + 1) * HF],
                                func=mybir.ActivationFunctionType.Copy,
                                scale=rcp[:, h : h + 1],
                            )
                            nc.scalar.activation(
                                out=t1[:, h * HF : (h + 1) * HF],
                                in_=u1c[:, j * C + h * HF : j * C + (h + 1) * HF],
                                func=mybir.ActivationFunctionType.Copy,
                                scale=rcp[:, 2 + h : 3 + h],
                            )
                        nc.vector.tensor_add(out=upd[:], in0=upd[:], in1=t1[:])
                        uT_ps = bps.tile([128, 128], f32, tag="uT_ps")
                        nc.tensor.transpose(
                            out=uT_ps[:], in_=upd[:], identity=ident_t[:]
                        )
                        uT = bsb.tile([128, 128], f32, tag="uT")
                        nc.vector.tensor_copy(out=uT[:], in_=uT_ps[:])
                        o_ps = bps.tile([128, 128], f32, tag="o_ps")
                        nc.tensor.matmul(
                            out=o_ps[:], lhsT=uT[:], rhs=w_sb["Wo"][:],
                            start=True, stop=True,
                        )
                        nc.vector.tensor_add(
                            out=obuf[:, j * 128 : (j + 1) * 128],
                            in0=o_ps[:],
                            in1=w_sb["boB"][:],
                        )
                    if DEBUG and cb == 0:
                        nc.sync.dma_start(out=dbg["u0c"][:, :], in_=u0c[:])
                        nc.sync.dma_start(out=dbg["ob0"][:, :], in_=obuf[:])
                    nc.sync.dma_start(
                        out=out[lo0 : lo0 + BB * 128, :].rearrange(
                            "(b p) c -> p b c", p=128
                        ),
                        in_=obuf[:],
                    )
    nc.finalize()
    return nc


def _get_nc():
    if "nc" not in _NC_CACHE:
        _NC_CACHE["nc"] = _build()
    return _NC_CACHE["nc"]


def kernel(**inputs):
    from concourse.bass_utils import run_bass_kernel_spmd

    w = _fold_weights(inputs)
    feat = np.ascontiguousarray(inputs["feat"], np.float32)
    shared = {
        "feat": feat,
        "AkX0": w["AkX0"], "AkX1": w["AkX1"],
        "AqX0": w["AqX0"], "AqX1": w["AqX1"],
        "Av0": w["Av0"], "Av1": w["Av1"],
        "Wo": w["Wo"], "boB": w["boB"],
        "iota": np.broadcast_to(
            np.arange(128, dtype=np.float32), (128, 128)
        ).copy(),
        "ident": np.eye(128, dtype=np.float32),
    }
    in_maps = []
    for i in range(NC):
        m = dict(shared)
        m.update(_prep_core(i, inputs))
        in_maps.append(m)

    nc = _get_nc()
    try:
        res = run_bass_kernel_spmd(nc, in_maps, list(range(NC)), trace=TRACE)
    except ModuleNotFoundError:
        res = run_bass_kernel_spmd(nc, in_maps, list(range(NC)))
    LAST["res"] = res
    out = np.concatenate(
        [res.results[i]["out"][:NPC] for i in range(NC)], axis=0
    )
    # host fix-up: nodes with no incoming edges in relation r lack that
    # relation's folded value-bias contribution
    for r, wv in ((0, w["wv0"]), (1, w["wv1"])):
        deg = np.bincount(inputs[f"dst{r}"], minlength=N)
        out[deg == 0] -= wv
    return out.astype(np.float32)


if __name__ == "__main__":
    import reference

    inp = {k: np.asarray(v) for k, v in reference.setup_inputs().items()}
    got = kernel(**inp)
    exp = np.asarray(reference.reference(**inp))
    err = np.abs(got - exp).max()
    rel = err / np.abs(exp).max()
    print("absmax err:", err, "rel:", rel)
